# revision 57
# baseline (speedup 1.0000x reference)
"""Distributed Trainium2 kernel for nn_CAnet (vq_codebook).

Sharding: 8 cores = 4 batches x 2 vertical halves (top/bottom). Each core
computes conv0 (3x3, 103->64), conv1 (3x3 d2), 2x2 avgpool and conv2 (3x3 d3)
for its half-image band (with halo rows), all as PSUM-accumulated bf16
matmuls. conv0/conv1 use output-column pair-packing: the PE stationary holds
128 columns = 64 out-channels x 2 output-column offsets, so each moving
element produces 128 outputs instead of 64 (~1.5x fewer PE cycles).
The cheap global stages (CBAM attention, soft-VQ encoding, bilinear
upsample, classifier) run host-side on the gathered c1/c2/c3.
"""

import numpy as np
import ml_dtypes

from concourse import bacc, mybir, tile
from concourse.bass_utils import run_bass_kernel_spmd

F32 = mybir.dt.float32
BF16 = mybir.dt.bfloat16

B = 4
CIN = 103
CF = 64
H = W = 256
# per-core local geometry (uniform for all cores), trimmed so every
# computed row is consumed: top core x rows [0,138), bottom [118,256)
XR = 138
C1R = 136
C2R = 132
PR = 66
C3R = 60
C1W, C2W, PW, C3W = 254, 250, 125, 119

_CACHE = {}
LAST_RESULT = None


FP8 = mybir.dt.float8e4
DR = mybir.MatmulPerfMode.DoubleRow
# conv0 runs as fp8 DoubleRow 3-term residual math at common scale 2^14:
#   T1 = (16*Whi)@xhi, T2 = Whi@xlo, T3 = Wlo@xhi,  c1 = relu(psum*2^-14 + b)
# with xhi=e4m3(16x), xlo=e4m3(256*(x-xhi/16)), Whi=e4m3(64w), Wlo=e4m3(1024*(w-Whi/64))
C0SCALE = 2.0 ** -14
# conv1: fp8 DoubleRow 2-term weight split on single-fp8 c1:
#   W1hi = e4m3(1024*w1), W1lo = e4m3(1024*(w1 - W1hi/1024)); psum = 2^10*c2pre
C1SCALE = 2.0 ** -10


def _build(nc):
    x_d = nc.dram_tensor("x", [CIN, 2, XR, W], FP8, kind="ExternalInput").ap()
    w12_d = nc.dram_tensor("w12", [CIN, 12, 2, 128], FP8, kind="ExternalInput").ap()
    w3_d = nc.dram_tensor("w3", [CIN, 6, 2, 128], FP8, kind="ExternalInput").ap()
    w1h_d = nc.dram_tensor("w1h", [CF, 6, 2, 128], FP8, kind="ExternalInput").ap()
    w1l_d = nc.dram_tensor("w1l", [CF, 6, 2, 128], FP8, kind="ExternalInput").ap()
    w2p_d = nc.dram_tensor("w2p", [2 * CF, 4, 128], BF16, kind="ExternalInput").ap()
    w2r_d = nc.dram_tensor("w2r", [CF, 4, 128], BF16, kind="ExternalInput").ap()
    b0_d = nc.dram_tensor("b0", [CF, 1], F32, kind="ExternalInput").ap()
    b1_d = nc.dram_tensor("b1", [CF, 1], F32, kind="ExternalInput").ap()
    b2_d = nc.dram_tensor("b2", [CF, 1], F32, kind="ExternalInput").ap()
    c1_d = nc.dram_tensor("c1", [CF, C1R, C1W], BF16, kind="ExternalOutput").ap()
    # c2 stored as two column planes (even cols, odd cols); host interleaves
    c2_d = nc.dram_tensor("c2", [CF, 2, C2R, PW], BF16, kind="ExternalOutput").ap()
    c3_d = nc.dram_tensor("c3", [CF, C3R, C3W], BF16, kind="ExternalOutput").ap()

    RELU = mybir.ActivationFunctionType.Relu

    with tile.TileContext(nc) as tc:
        with (
            tc.tile_pool(name="wpool", bufs=1) as wpool,
            tc.tile_pool(name="xb", bufs=3) as xbp,
            tc.tile_pool(name="c1o", bufs=2) as c1op,
            tc.tile_pool(name="act", bufs=1) as actp,
            tc.tile_pool(name="c3t", bufs=2) as c3p,
            tc.tile_pool(name="ps", bufs=2, space="PSUM") as psp,
            tc.tile_pool(name="ps1", bufs=2, space="PSUM") as psp1,
            tc.tile_pool(name="ps2", bufs=2, space="PSUM") as psp2,
        ):
            # weights: conv0/conv1 fp8 direct; conv2 bf16 direct
            w12t = wpool.tile([CIN, 12, 2, 128], FP8, tag="w12t")
            w3t = wpool.tile([CIN, 6, 2, 128], FP8, tag="w3t")
            w1ht = wpool.tile([CF, 6, 2, 128], FP8, tag="w1ht")
            w1lt = wpool.tile([CF, 6, 2, 128], FP8, tag="w1lt")
            w2pb = wpool.tile([2 * CF, 4, 128], BF16, tag="w2pb")
            w2rb = wpool.tile([CF, 4, 128], BF16, tag="w2rb")
            b0s = wpool.tile([CF, 1], F32, tag="b0s")
            b1s = wpool.tile([CF, 1], F32, tag="b1s")
            b2s = wpool.tile([CF, 1], F32, tag="b2s")
            # conv0's weights first so its first matmul starts ASAP; the
            # conv1/conv2 weight loads are emitted inside the conv0 loop.
            x0b = xbp.tile([CIN, 2, 10, W], FP8, tag="xb")
            nc.sync.dma_start(out=x0b[:, :, 0:4, :], in_=x_d[:, :, 0:4, :])
            nc.sync.dma_start(out=w3t[:], in_=w3_d[:])
            nc.sync.dma_start(out=w12t[:], in_=w12_d[:])

            later_loads = {
                16: [(w1ht, w1h_d)],
                32: [(w1lt, w1l_d), (b1s, b1_d)],
                48: [(w2pb, w2p_d)],
                64: [(w2rb, w2r_d), (b2s, b2_d)],
            }

            # persistent activation tiles; c1f is the fp8 copy of c1 consumed
            # by conv1 (cols 254/255 are zero padding so every DR can run the
            # full 63-wide moving range)
            c1f = actp.tile([CF, C1R, 256], FP8, tag="c1f")
            c2e = actp.tile([CF, C2R, PW], BF16, tag="c2e")
            c2o = actp.tile([CF, C2R, PW], BF16, tag="c2o")
            pls = actp.tile([2 * CF, PR, PW], BF16, tag="pls")
            nc.gpsimd.memset(c1f[:, :, 254:256], 0.0)

            # ---- conv0: 3x3 valid 103->64, fp8 DoubleRow 3-term, col-pair
            # packed (M=128). 18 DR per 2-row group: 12 tap-paired (T1 hi,
            # T2 lo) + 6 Wlo (T3) with (dy0,dy2) row-split / (dy1 s,s+1)
            # col-split plane pairing.
            for r0 in range(0, C1R, 8):
                rows = min(8, C1R - r0)
                if r0 == 0:
                    # first 4 rows were DMA'd before the weights
                    xb = x0b
                    nc.sync.dma_start(out=xb[:, :, 4:rows + 2, :],
                                      in_=x_d[:, :, 4:rows + 2, :])
                    nc.sync.dma_start(out=b0s[:, :], in_=b0_d[:, :])
                else:
                    xb = xbp.tile([CIN, 2, 10, W], FP8, tag="xb")
                    nc.sync.dma_start(out=xb[:, :, 0:rows + 2, :],
                                      in_=x_d[:, :, r0:r0 + rows + 2, :])
                for dst, src in later_loads.get(r0, ()):
                    if dst.shape[-1] == 1:
                        nc.sync.dma_start(out=dst[:, :], in_=src[:, :])
                    else:
                        nc.sync.dma_start(out=dst[:], in_=src[:])
                c1o = c1op.tile([CF, 8, C1W], BF16, tag="c1o")

                def _t3(ps, g, rr):
                    # T3 row-pairs: plane0 = (dy0,s), plane1 = (dy2,s)
                    for s in range(4):
                        nc.tensor.matmul(
                            out=ps[:, g:g + 2, :],
                            lhsT=w3t[:, s, :, :],
                            rhs=xb[:, 0, rr:rr + 4, s:min(s + 254, W):2]
                                .rearrange("p (a b) j -> p a b j", a=2),
                            start=(s == 0), stop=False,
                            perf_mode=DR,
                        )
                    # T3 dy1 col-pairs: planes = (dy1,s), (dy1,s+1)
                    for si in range(2):
                        s = 2 * si
                        nc.tensor.matmul(
                            out=ps[:, g:g + 2, :],
                            lhsT=w3t[:, 4 + si, :, :],
                            rhs=xb[:, 0, rr + 1:rr + 3, s:s + 254]
                                .rearrange("p r (j two) -> p two r j", two=2),
                            start=False, stop=False,
                            perf_mode=DR,
                        )

                def _t12(ps, g, rr):
                    # T1+T2: plane0 = 16Whi @ xhi, plane1 = Whi @ xlo
                    k = 0
                    for dy in range(3):
                        for s in range(4):
                            nc.tensor.matmul(
                                out=ps[:, g:g + 2, :],
                                lhsT=w12t[:, dy * 4 + s, :, :],
                                rhs=xb[:, :, rr + dy:rr + dy + 2,
                                       s:min(s + 254, W):2],
                                start=False, stop=(k == 11),
                                perf_mode=DR,
                            )
                            k += 1

                def _evac(ps, hf, hr):
                    for par in range(2):
                        nc.scalar.activation(
                            out=c1o[:, hf:hf + hr, par:254:2],
                            in_=ps[par * CF:(par + 1) * CF, 0:hr, :],
                            func=RELU, bias=b0s[:, 0:1], scale=C0SCALE,
                        )

                if False:
                    pass
                else:
                    for hf in range(0, rows, 4):
                        hr = min(4, rows - hf)
                        ps = psp.tile([2 * CF, 4, 127], F32, tag="ps")
                        for g in range(0, hr, 2):
                            _t3(ps, g, hf + g)
                            _t12(ps, g, hf + g)
                        _evac(ps, hf, hr)
                # fp8 copy of c1 for conv1, alternating DVE/GPSIMD (the
                # scalar engine is near-saturated with PSUM evacuations)
                eng = (nc.vector, nc.gpsimd)[(r0 // 8) % 2]
                eng.tensor_copy(out=c1f[:, r0:r0 + rows, 0:254],
                                in_=c1o[:, 0:rows, :])
                nc.sync.dma_start(out=c1_d[:, r0:r0 + rows, :],
                                  in_=c1o[:, 0:rows, :])

            # ---- conv2 block emitter (interleaved into conv1 loop) ----
            # split emission: _conv2_mm_p (K=128 pool+shift reads, start) can
            # run a conv1-block earlier than _conv2_mm_r (dy2 reads of the
            # freshest pool rows, stop) + evac
            def _conv2_block(r2, mm_p_done=False):
                rows = min(8, C3R - r2)
                ps = psp2.tile([2 * CF, 8, 3, 20], F32, tag="ps2")
                for cp in range(3):
                    bn_ = 20 if cp < 2 else 19
                    for si in range(4):
                        jn = bn_ if si == 3 else 20
                        base = cp + 3 * si
                        nc.tensor.matmul(
                            out=ps[:, 0:rows, cp, 0:jn],
                            lhsT=w2pb[:, si, :],
                            rhs=pls[:, r2:r2 + rows,
                                    base:min(base + 6 * jn, PW):6],
                            start=(si == 0), stop=False,
                        )
                        nc.tensor.matmul(
                            out=ps[:, 0:rows, cp, 0:jn],
                            lhsT=w2rb[:, si, :],
                            rhs=pls[0:CF, r2 + 6:r2 + 6 + rows,
                                    base:min(base + 6 * jn, PW):6],
                            start=False, stop=(si == 3),
                        )
                c3t = c3p.tile([CF, 8, 126], BF16, tag="c3t")

                def _cols(base, ncp, nj, rr=rows):
                    a = c3t[:, 0:rr, base:base + 6 * nj]
                    a = a.rearrange("p r (j s) -> p r j s", s=6)
                    return a[:, :, :, 0:ncp].transpose([0, 1, 3, 2])

                nc.scalar.activation(
                    out=_cols(0, 3, 20), in_=ps[0:CF, 0:rows, :, :],
                    func=RELU, bias=b2s[:, 0:1],
                )
                nc.scalar.activation(
                    out=_cols(3, 2, 20), in_=ps[CF:2 * CF, 0:rows, 0:2, :],
                    func=RELU, bias=b2s[:, 0:1],
                )
                nc.scalar.activation(
                    out=_cols(5, 1, 19), in_=ps[CF:2 * CF, 0:rows, 2, 0:19],
                    func=RELU, bias=b2s[:, 0:1],
                )
                nc.sync.dma_start(out=c3_d[:, r2:r2 + rows, :],
                                  in_=c3t[:, 0:rows, 0:C3W])

            # ---- conv1: 3x3 d2 valid 64->64, col-pair packed ----
            # out cols (4j+p, 4j+2+p) for p in {0,1}; moving col = p + s + 4j
            next_r2 = 0
            for r1 in range(0, C2R, 8):
                rows = min(8, C2R - r1)
                # emit conv2 blocks ready from the PREVIOUS block's pool rows
                # so their matmuls queue on PE without waiting on this
                # block's evac/pool chain
                # (the last two conv2 blocks are held for the tail flush so
                # conv2(48)'s matmuls keep PE busy while conv2(56)'s pool
                # dependency chain drains)
                while next_r2 < C3R - 16 and r1 // 2 >= min(next_r2 + 14, PR):
                    _conv2_block(next_r2)
                    next_r2 += 8
                ps1a = psp1.tile([2 * CF, 8, 63], F32, tag="ps1a")
                ps1b = psp1.tile([2 * CF, 8, 63], F32, tag="ps1b")
                pst = [ps1a, ps1b]
                # fp8 DoubleRow 2-term: 12 DR per 4-row group per p.
                # w1h/w1l pair slots: si<4: planes (dy0,si),(dy2,si) via
                # 8-row split; 4+sk: planes (dy1,2sk),(dy1,2sk+1) via col split
                for p in range(2):
                    for sub in range(0, rows, 4):
                        rr = r1 + sub
                        k = 0
                        for wt in (w1ht, w1lt):
                            for si in range(4):
                                nc.tensor.matmul(
                                    out=pst[p][:, sub:sub + 4, 0:63],
                                    lhsT=wt[:, si, :, :],
                                    rhs=c1f[:, rr:rr + 8,
                                            p + 2 * si:p + 2 * si + 249:4]
                                        .rearrange("c (a b) j -> c a b j",
                                                   a=2),
                                    start=(k == 0), stop=False,
                                    perf_mode=DR,
                                )
                                k += 1
                            for sk in range(2):
                                base = p + 4 * sk
                                nc.tensor.matmul(
                                    out=pst[p][:, sub:sub + 4, 0:63],
                                    lhsT=wt[:, 4 + sk, :, :],
                                    rhs=c1f[:, rr + 2:rr + 6,
                                            base:base + 251:2]
                                        .rearrange("c r (j two) -> c two r j",
                                                   two=2),
                                    start=False, stop=(k == 11),
                                    perf_mode=DR,
                                )
                                k += 1
                # evacuate: A cols 4j+p -> plane[p][2j], B cols 4j+2+p -> plane[p][2j+1]
                for p in range(2):
                    plane = c2e if p == 0 else c2o
                    nc.scalar.activation(
                        out=plane[:, r1:r1 + rows, 0:125:2],
                        in_=pst[p][0:CF, 0:rows, :],
                        func=RELU, bias=b1s[:, 0:1], scale=C1SCALE,
                    )
                    nc.scalar.activation(
                        out=plane[:, r1:r1 + rows, 1:124:2],
                        in_=pst[p][CF:2 * CF, 0:rows, 0:62],
                        func=RELU, bias=b1s[:, 0:1], scale=C1SCALE,
                    )
                nc.sync.dma_start(out=c2_d[:, 0, r1:r1 + rows, :],
                                  in_=c2e[:, r1:r1 + rows, :])
                nc.sync.dma_start(out=c2_d[:, 1, r1:r1 + rows, :],
                                  in_=c2o[:, r1:r1 + rows, :])
                # pool rows q0..q0+rows/2 (sum only; 0.25 folded into w2)
                q0 = r1 // 2
                qr = rows // 2
                nc.vector.tensor_add(
                    out=pls[0:CF, q0:q0 + qr, :],
                    in0=c2e[:, r1:r1 + rows:2, :],
                    in1=c2e[:, r1 + 1:r1 + rows:2, :],
                )
                pscr = c3p.tile([CF, 4, PW], BF16, tag="pscr")
                nc.vector.tensor_add(
                    out=pscr[:, 0:qr, :],
                    in0=c2o[:, r1:r1 + rows:2, :],
                    in1=c2o[:, r1 + 1:r1 + rows:2, :],
                )
                nc.vector.tensor_add(
                    out=pls[0:CF, q0:q0 + qr, :],
                    in0=pls[0:CF, q0:q0 + qr, :],
                    in1=pscr[:, 0:qr, :],
                )
                # incremental shifted copy: pls[64+ch, q] = pool[ch, q+3]
                slo = max(q0, 3)
                if q0 + qr > slo:
                    nc.vector.tensor_copy(
                        out=pls[CF:2 * CF, slo - 3:q0 + qr - 3, :],
                        in_=pls[0:CF, slo:q0 + qr, :],
                    )
                # tail: at the second-to-last conv1 block emit conv2(48) and
                # the pool-half of conv2(56); after the last block only the
                # dy2 matmuls of conv2(56) remain on the critical chain
                if r1 + 16 >= C2R and r1 + 8 < C2R:
                    while next_r2 < C3R - 8:
                        _conv2_block(next_r2)
                        next_r2 += 8
                elif r1 + 8 >= C2R:
                    while next_r2 < C3R:
                        _conv2_block(next_r2)
                        next_r2 += 8
    nc.compile()
    return nc


def _get_compiled():
    if "nc" not in _CACHE:
        _CACHE["nc"] = _build(
            bacc.Bacc("TRN2", target_bir_lowering=False, debug=False, num_devices=8)
        )
    return _CACHE["nc"]


def _prep_weights(conv0_w, conv1_w, conv2_w):
    E4 = ml_dtypes.float8_e4m3
    # conv0 hi/lo split at scales: Whi ~ 64w, Wlo ~ 1024*(w - Whi/64)
    w0hi = np.asarray(64.0 * conv0_w, E4)
    w0hi_f = w0hi.astype(np.float32)
    w0lo = np.asarray(16.0 * (64.0 * conv0_w - w0hi_f), E4).astype(np.float32)
    w0hi16 = 16.0 * w0hi_f

    def _pack(wsrc, dy, s):
        # A/B col-pair packing for moving col offset s: [cin, 128]
        out = np.zeros((CIN, 128), np.float32)
        if s <= 2:
            out[:, 0:CF] = wsrc[:, :, dy, s].T
        if s >= 1:
            out[:, CF:128] = wsrc[:, :, dy, s - 1].T
        return out

    # w12: [cin, tap, plane(hi16, hi), 128]
    w12 = np.zeros((CIN, 12, 2, 128), np.float32)
    for dy in range(3):
        for s in range(4):
            w12[:, dy * 4 + s, 0] = _pack(w0hi16, dy, s)
            w12[:, dy * 4 + s, 1] = _pack(w0hi_f, dy, s)
    # w3: Wlo planes; pairs 0-3: (dy0,s),(dy2,s); pairs 4-5: (dy1,s),(dy1,s+1)
    w3 = np.zeros((CIN, 6, 2, 128), np.float32)
    for s in range(4):
        w3[:, s, 0] = _pack(w0lo, 0, s)
        w3[:, s, 1] = _pack(w0lo, 2, s)
    for si in range(2):
        w3[:, 4 + si, 0] = _pack(w0lo, 1, 2 * si)
        w3[:, 4 + si, 1] = _pack(w0lo, 1, 2 * si + 1)
    w12 = np.asarray(w12, E4)
    w3 = np.asarray(w3, E4)
    # conv1: fp8 2-term weight split; per-tap A/B col-pair packing
    # (A: out 4j+p gets dx=si; B: out 4j+2+p gets dx=si-1)
    w1hi = np.asarray(1024.0 * conv1_w, E4)
    w1hi_f = w1hi.astype(np.float32)
    w1lo = np.asarray(1024.0 * (conv1_w - w1hi_f / 1024.0), E4) \
        .astype(np.float32)

    def _pack1(wsrc, dy, si):
        out = np.zeros((CF, 128), np.float32)
        if si <= 2:
            out[:, 0:CF] = wsrc[:, :, dy, si].T
        if si >= 1:
            out[:, CF:128] = wsrc[:, :, dy, si - 1].T
        return out

    def _mk_w1(wsrc):
        t = np.zeros((CF, 6, 2, 128), np.float32)
        for si in range(4):
            t[:, si, 0] = _pack1(wsrc, 0, si)
            t[:, si, 1] = _pack1(wsrc, 2, si)
        for sk in range(2):
            t[:, 4 + sk, 0] = _pack1(wsrc, 1, 2 * sk)
            t[:, 4 + sk, 1] = _pack1(wsrc, 1, 2 * sk + 1)
        return np.asarray(t, E4)

    w1h = _mk_w1(w1hi_f)
    w1l = _mk_w1(w1lo)
    # conv2: col-pair packed like conv1 (A: dx=si, B: dx=si-1),
    # 0.25 pool-average folded in
    w2p = np.zeros((2 * CF, 4, 128), np.float32)
    w2r = np.zeros((CF, 4, 128), np.float32)
    for si in range(4):
        if si <= 2:
            w2p[0:CF, si, 0:CF] = 0.25 * conv2_w[:, :, 0, si].T
            w2p[CF:, si, 0:CF] = 0.25 * conv2_w[:, :, 1, si].T
            w2r[:, si, 0:CF] = 0.25 * conv2_w[:, :, 2, si].T
        if si >= 1:
            w2p[0:CF, si, CF:128] = 0.25 * conv2_w[:, :, 0, si - 1].T
            w2p[CF:, si, CF:128] = 0.25 * conv2_w[:, :, 1, si - 1].T
            w2r[:, si, CF:128] = 0.25 * conv2_w[:, :, 2, si - 1].T
    w2p = np.asarray(w2p, ml_dtypes.bfloat16)
    w2r = np.asarray(w2r, ml_dtypes.bfloat16)
    return w12, w3, w1h, w1l, w2p, w2r


def _upsample(x, Ho, Wo):
    """bilinear, align_corners=True, float32"""
    def mat1(n_out, n_in):
        idx = np.arange(n_out, dtype=np.float64) * ((n_in - 1) / (n_out - 1))
        lo = np.clip(np.floor(idx).astype(np.int64), 0, n_in - 2)
        f = (idx - lo).astype(np.float32)
        return lo, f
    lo, f = mat1(Ho, x.shape[2])
    x = x[:, :, lo, :] * (1.0 - f)[None, None, :, None] \
        + x[:, :, lo + 1, :] * f[None, None, :, None]
    lo, f = mat1(Wo, x.shape[3])
    x = x[:, :, :, lo] * (1.0 - f)[None, None, None, :] \
        + x[:, :, :, lo + 1] * f[None, None, None, :]
    return x.astype(np.float32)


def kernel(x, conv0_w, conv0_b, conv1_w, conv1_b, conv2_w, conv2_b, convp_w,
           fc1_w, fc2_w, enc_w, codewords, scale, attn_w, attn_b,
           bn_w, bn_b, cls_w, cls_b):
    global LAST_RESULT
    x = np.asarray(x, np.float32)
    to32 = lambda a: np.asarray(a, np.float32)
    (conv0_w, conv0_b, conv1_w, conv1_b, conv2_w, conv2_b, convp_w, fc1_w,
     fc2_w, enc_w, codewords, scale, attn_w, attn_b, bn_w, bn_b, cls_w,
     cls_b) = map(to32, (conv0_w, conv0_b, conv1_w, conv1_b, conv2_w, conv2_b,
                         convp_w, fc1_w, fc2_w, enc_w, codewords, scale,
                         attn_w, attn_b, bn_w, bn_b, cls_w, cls_b))

    nc = _get_compiled()
    w12, w3, w1h, w1l, w2p, w2r = _prep_weights(conv0_w, conv1_w, conv2_w)
    common = {
        "w12": w12, "w3": w3, "w1h": w1h, "w1l": w1l, "w2p": w2p, "w2r": w2r,
        "b0": conv0_b.reshape(CF, 1).copy(),
        "b1": conv1_b.reshape(CF, 1).copy(),
        "b2": conv2_b.reshape(CF, 1).copy(),
    }
    E4 = ml_dtypes.float8_e4m3
    xhi = np.asarray(16.0 * x, E4)
    xlo = np.asarray(16.0 * (16.0 * x - xhi.astype(np.float32)), E4)
    in_maps = []
    for i in range(8):
        b, h = i // 2, i % 2
        r0 = 0 if h == 0 else H - XR
        xs = np.stack([xhi[b, :, r0:r0 + XR, :], xlo[b, :, r0:r0 + XR, :]],
                      axis=1)
        in_maps.append({"x": np.ascontiguousarray(xs), **common})

    res = run_bass_kernel_spmd(nc, in_maps, core_ids=list(range(8)))
    LAST_RESULT = res

    c1 = np.empty((B, CF, 254, 254), np.float32)
    c2 = np.empty((B, CF, 250, 250), np.float32)
    c3 = np.empty((B, CF, 119, 119), np.float32)
    for i in range(8):
        b, h = i // 2, i % 2
        r = res.results[i]
        s1 = np.asarray(r["c1"]).astype(np.float32)
        s2 = np.asarray(r["c2"]).astype(np.float32)
        s2f = np.empty((CF, C2R, C2W), np.float32)
        s2f[:, :, 0::2] = s2[:, 0]
        s2f[:, :, 1::2] = s2[:, 1]
        s3 = np.asarray(r["c3"]).astype(np.float32)
        if h == 0:
            c1[b, :, 0:127] = s1[:, 0:127]
            c2[b, :, 0:125] = s2f[:, 0:125]
            c3[b, :, 0:60] = s3[:, 0:60]
        else:
            c1[b, :, 127:254] = s1[:, 9:136]
            c2[b, :, 125:250] = s2f[:, 7:132]
            c3[b, :, 60:119] = s3[:, 1:60]

    # ---- host: CBAM channel attention ----
    sig = lambda v: 1.0 / (1.0 + np.exp(-v))

    def fc(v):
        return np.maximum(v @ fc1_w.T, 0.0) @ fc2_w.T

    gate_c = sig(fc(c3.mean((2, 3))) + fc(c3.max((2, 3))))
    xg = gate_c[:, :, None, None] * c3
    # ---- spatial attention: 7x7 conv, pad 3, 2 -> 1 channel ----
    sp = np.stack([xg.mean(1), xg.max(1)], 1)          # (B,2,119,119)
    pad = convp_w.shape[-1] // 2
    spp = np.pad(sp, ((0, 0), (0, 0), (pad, pad), (pad, pad)))
    conv_sp = np.zeros((B, C3W, C3W), np.float32)
    for c in range(2):
        for dy in range(7):
            for dx in range(7):
                conv_sp += convp_w[0, c, dy, dx] * \
                    spp[:, c, dy:dy + C3W, dx:dx + C3W]
    x_p = sig(conv_sp)[:, None, :, :] * c3
    xx = c3 + x_p
    # ---- soft-VQ context encoding (batch folded into n, as in reference) ----
    K, D = codewords.shape
    feat = np.maximum(np.tensordot(enc_w, xx, axes=([1], [1])), 0.0)
    feat = feat.transpose(1, 0, 2, 3)                  # (B, D, h, w)
    Zf = feat.reshape(1, D, -1)[0].T                   # (n, D)
    norm = ((Zf ** 2).sum(-1, keepdims=True)
            + (codewords ** 2).sum(-1)[None, :]
            - 2.0 * Zf @ codewords.T)
    logit = scale[None, :] * norm
    logit -= logit.max(1, keepdims=True)
    Aexp = np.exp(logit)
    A = Aexp / Aexp.sum(1, keepdims=True)              # (n, K)
    E = A.T @ Zf - A.sum(0)[:, None] * codewords       # (K, D)
    mu = E.mean(1, keepdims=True)
    var = ((E - mu) ** 2).mean(1, keepdims=True)
    En = (E - mu) / np.sqrt(var + 1e-5) * bn_w[:, None] + bn_b[:, None]
    E_sum = np.maximum(En, 0.0).sum(0)[None, :]        # (1, D)
    gamma = sig(E_sum @ attn_w.T + attn_b).reshape(-1, CF, 1, 1)
    xx = xx + xx * gamma
    # ---- upsample + concat + classifier ----
    cat = np.concatenate(
        [_upsample(c1, H, W), _upsample(c2, H, W), _upsample(xx, H, W)], 1)
    out = np.tensordot(cls_w, cat, axes=([1], [1])).transpose(1, 0, 2, 3)
    out = out + cls_b[None, :, None, None]
    return np.ascontiguousarray(out.astype(np.float32))

